# revision 10
# baseline (speedup 1.0000x reference)
"""Distributed cosine-similarity kNN retrieval (EpisodicSDM) on 8 Trainium2 cores.

Reference computation:
    x_norm = normalize(x); k_norm = normalize(keys)
    scores = x_norm @ k_norm.T               [B, N]
    top_vals, top_idx = top_k(scores, 8)
    out = sum_k softmax(top_vals)_k * values[top_idx_k]

Two SPMD dispatches, no collectives:

Dispatch A (keys sharded along N, all queries everywhere):
  - per 128-key tile: row norms via one fused ACT Square+accumulate, then a
    PE fp32 matmul against diag(1/||k||) = normalized transpose -> kT bf16
  - coarse scores S = x_norm @ k_norm.T in bf16 (fp32 PSUM accumulate),
    26 PSUM n-tiles per 128-query tile
  - fold1 (pairwise max of adjacent PSUM n-tiles): ACT copies the even
    tile to SBUF, DVE tensor_tensor max reads the odd tile from PSUM
    (only one PSUM operand is legal) -> m1 bf16 [128, 6656]
  - fold2 on GPSIMD (adjacent m1 blocks) -> m2 bf16 [128, 3584]
  - ACT widens m2 to fp32, DVE ORs the slot index into the low mantissa
    bits, max8 -> top-8 packed (value, slot) per core, tie-free, no
    max_index pass.
  -> outputs: packed candidates [B, 8] + per-key 1/||k|| [NLOC].

Host glue: concatenate candidates -> [B, 64]; append the device-computed
inverse norms as column 256 of the padded key table (pure data movement).

Dispatch B (queries sharded, 256 per core):
  - re-pack: strip slot bits, embed position 0..63 for tie-free pruning
  - top-12 slots of 64 via max8 + match_replace + max8
  - slot -> 4 member keys: row = core*13312 + 2048*(j>>9) + (j&511) + 512*m
    (leftover fold block: members 2 and 3 are clamped to a pad row)
  - gather member rows [257] (key + inv norm) via indirect DMA
  - exact fp32 rescore s = <x_norm, k> * invnorm_k; top-8 of 48; softmax;
    indirect-gather value rows; weighted sum -> [256, 256] slice.
"""

import os
import sys
import time

_TRN_REPO = "/opt/trn_rl_repo"
if _TRN_REPO not in sys.path:
    sys.path.insert(0, _TRN_REPO)

import numpy as np

import concourse.bass as bass
import concourse.mybir as mybir
import concourse.tile as tile
from concourse import bacc
from concourse.bass import IndirectOffsetOnAxis
from concourse.bass_utils import run_bass_kernel_spmd
from concourse.masks import make_identity

F32 = mybir.dt.float32
BF16 = mybir.dt.bfloat16
I32 = mybir.dt.int32
U32 = mybir.dt.uint32
ALU = mybir.AluOpType
ACTF = mybir.ActivationFunctionType
AX = mybir.AxisListType

# ---- problem constants ----
B = 2048
D = 256
N = 100000
TOPK = 8
NCORES = 8
NLOC = 14336              # 28 * 512 per-core shard; 8*14336 = 114688 >= N
NPAD = NLOC * NCORES
NT = 512
BSLOTS = 12               # slots kept per query after the cross-core merge

_PACK_MASK = 0x1FFF
_NEG_BIG = -3.0e38


def _fold_geom(nloc, nt=NT):
    ntiles = nloc // nt
    assert ntiles % 4 == 0
    ngrp = ntiles // 4           # quad groups: 2048 keys -> 1024 m1 cells
    m1w = ngrp * 2 * nt          # fold output width (nloc / 2)
    return ntiles, ngrp, m1w


# --------------------------------------------------------------------------
# Dispatch A
# --------------------------------------------------------------------------

def build_dispatch_a(bq=B, nloc=NLOC, nt=NT, dbg=False):
    ntiles, ngrp, m1w = _fold_geom(nloc, nt)
    qtiles = bq // 128
    ktiles = nloc // 128
    assert m1w <= _PACK_MASK + 1 and m1w >= 8
    assert ktiles % 4 == 0

    nc = bacc.Bacc("TRN2", target_bir_lowering=False, debug=dbg)
    x_d = nc.dram_tensor("x", [bq, D], F32, kind="ExternalInput").ap()
    k_d = nc.dram_tensor("keys", [nloc, D], F32, kind="ExternalInput").ap()
    out_d = nc.dram_tensor("cand", [bq, 8], F32, kind="ExternalOutput").ap()
    kinv_d = nc.dram_tensor("kinv", [nloc, 1], F32, kind="ExternalOutput").ap()

    with tile.TileContext(nc) as tc:
        with (
            tc.tile_pool(name="const", bufs=1) as constp,
            tc.tile_pool(name="kprep", bufs=4) as kprep,
            tc.tile_pool(name="big", bufs=1) as bigp,
            tc.tile_pool(name="xp", bufs=2) as xp,
            tc.tile_pool(name="sp", bufs=4) as sp,
            tc.tile_pool(name="mp", bufs=1) as mp,
            tc.tile_pool(name="ps", bufs=2, space="PSUM") as psp,
        ):
            identb = constp.tile([128, 128], BF16)
            make_identity(nc, identb[:])
            identf = constp.tile([128, 128], F32)
            make_identity(nc, identf[:])
            eps = constp.tile([128, 1], F32)
            nc.gpsimd.memset(eps[:], 1e-30)
            iota_pack = constp.tile([128, m1w], I32)
            nc.gpsimd.iota(iota_pack[:], pattern=[[1, m1w]], base=0,
                           channel_multiplier=0)
            maskc = constp.tile([128, 1], I32)
            nc.gpsimd.memset(maskc[:], -(_PACK_MASK + 1))

            kT = bigp.tile([128, 2, nloc], BF16)
            xT = bigp.tile([128, 2, bq], BF16)

            # ---- x prep ----
            for qt in range(qtiles):
                xt = xp.tile([128, D], F32, tag="xt")
                nc.sync.dma_start(out=xt[:], in_=x_d[qt * 128:(qt + 1) * 128, :])
                xsq = xp.tile([128, D], F32, tag="xsq")
                xn2 = xp.tile([128, 1], F32, tag="xn2")
                nc.scalar.activation(xsq[:], xt[:], ACTF.Square, accum_out=xn2[:])
                xsrt = xp.tile([128, 1], F32, tag="xsrt")
                nc.scalar.activation(xsrt[:], xn2[:], ACTF.Sqrt)
                xinv = xp.tile([128, 1], F32, tag="xinv")
                nc.vector.reciprocal(xinv[:], xsrt[:])
                xnb = xp.tile([128, D], BF16, tag="xnb")
                nc.scalar.activation(xnb[:], xt[:], ACTF.Copy, scale=xinv[:])
                for c in range(2):
                    pt = psp.tile([128, 2 * nt], F32, tag="pa")
                    nc.tensor.matmul(pt[:, :128], lhsT=xnb[:, c * 128:(c + 1) * 128],
                                     rhs=identb[:], start=True, stop=True)
                    nc.scalar.activation(xT[:, c, qt * 128:(qt + 1) * 128],
                                         pt[:, :128], ACTF.Copy)

            # ---- key prep: groups of 4 tiles -> one PSUM bank per chunk ----
            kinv_all = bigp.tile([128, ktiles], F32)
            for g in range(ktiles // 4):
                ktfs = []
                kn2g = kprep.tile([128, 4], F32, tag="kn2g")
                for i in range(4):
                    tk = g * 4 + i
                    ktf = kprep.tile([128, D], F32, tag=f"ktf{i}")
                    nc.sync.dma_start(out=ktf[:],
                                      in_=k_d[tk * 128:(tk + 1) * 128, :])
                    ksq = kprep.tile([128, D], F32, tag=f"ksq{i}")
                    nc.scalar.activation(ksq[:], ktf[:], ACTF.Square,
                                         accum_out=kn2g[:, i:i + 1])
                    ktfs.append(ktf)
                ksrt = kprep.tile([128, 4], F32, tag="ksrt")
                nc.scalar.activation(ksrt[:], kn2g[:], ACTF.Sqrt, bias=eps[:])
                kinvg = kprep.tile([128, 4], F32, tag="kinvg")
                nc.vector.reciprocal(kinvg[:], ksrt[:])
                nc.vector.tensor_copy(kinv_all[:, g * 4:(g + 1) * 4], kinvg[:])
                diags = []
                for i in range(4):
                    diag = kprep.tile([128, 128], F32, tag=f"diag{i}")
                    nc.gpsimd.tensor_tensor(
                        diag[:], identf[:],
                        kinvg[:, i:i + 1].to_broadcast([128, 128]),
                        op=ALU.mult)
                    diags.append(diag)
                for c in range(2):
                    pt = psp.tile([128, 2 * nt], F32, tag="pa")
                    for i in range(4):
                        nc.tensor.matmul(
                            pt[:, i * 128:(i + 1) * 128],
                            lhsT=ktfs[i][:, c * 128:(c + 1) * 128],
                            rhs=diags[i][:], start=True, stop=True)
                    nc.scalar.activation(kT[:, c, g * nt:(g + 1) * nt],
                                         pt[:, :nt], ACTF.Copy)
            # kinv [128, ktiles] -> DRAM [nloc, 1] (key id = tk*128 + p)
            nc.sync.dma_start(
                out=kinv_d[:].rearrange("(t p) o -> p (t o)", p=128),
                in_=kinv_all[:])

            # ---- main loop: quad groups, single fold to packed fp32 ----
            for qt in range(qtiles):
                m1f = mp.tile([128, m1w], F32, tag="m1f")
                for g in range(ngrp):
                    pa = psp.tile([128, 2 * nt], F32, tag="pa")
                    pb = psp.tile([128, 2 * nt], F32, tag="pb")
                    for idx, pp in enumerate((pa, pa, pb, pb)):
                        nti = 4 * g + idx
                        dst = pp[:, (idx % 2) * nt:(idx % 2 + 1) * nt]
                        for c in range(2):
                            nc.tensor.matmul(
                                dst,
                                lhsT=xT[:, c, qt * 128:(qt + 1) * 128],
                                rhs=kT[:, c, nti * nt:(nti + 1) * nt],
                                start=(c == 0), stop=(c == 1))
                    stmp = sp.tile([128, 2 * nt], F32, tag="stmp")
                    nc.scalar.activation(stmp[:], pa[:], ACTF.Copy)
                    nc.vector.tensor_tensor(
                        m1f[:, g * 2 * nt:(g + 1) * 2 * nt],
                        pb[:], stmp[:], op=ALU.max)
                nc.vector.scalar_tensor_tensor(
                    m1f[:].bitcast(I32), m1f[:].bitcast(I32),
                    maskc[:], iota_pack[:],
                    op0=ALU.bitwise_and, op1=ALU.bitwise_or)
                top = mp.tile([128, 8], F32, tag="top")
                nc.vector.max(out=top[:], in_=m1f[:])
                nc.sync.dma_start(out=out_d[qt * 128:(qt + 1) * 128, :],
                                  in_=top[:])

    nc.compile()
    return nc


# --------------------------------------------------------------------------
# Dispatch B
# --------------------------------------------------------------------------

def build_dispatch_b(bq_slice, nloc=NLOC, npad=NPAD, ncand=NCORES * 8,
                     bslots=BSLOTS, dbg=False):
    qtiles = bq_slice // 128
    ntiles, ngrp, m1w = _fold_geom(nloc)
    nmemb = bslots * 2
    DA = D + 1   # augmented row: 256 dims + inv-norm

    nc = bacc.Bacc("TRN2", target_bir_lowering=False, debug=dbg)
    v_d = nc.dram_tensor("vals", [bq_slice, ncand], F32, kind="ExternalInput").ap()
    x_d = nc.dram_tensor("x", [bq_slice, D], F32, kind="ExternalInput").ap()
    k_d = nc.dram_tensor("keysaug", [npad, DA], F32, kind="ExternalInput").ap()
    val_d = nc.dram_tensor("values", [npad, D], F32, kind="ExternalInput").ap()
    out_d = nc.dram_tensor("out", [bq_slice, D], F32, kind="ExternalOutput").ap()

    with tile.TileContext(nc) as tc:
        with (
            tc.tile_pool(name="const", bufs=1) as constp,
            tc.tile_pool(name="wp", bufs=2) as wp,
            tc.tile_pool(name="gp", bufs=2) as gp,
        ):
            iota_cand_i = constp.tile([128, ncand], I32)
            nc.gpsimd.iota(iota_cand_i[:], pattern=[[1, ncand]], base=0,
                           channel_multiplier=0)
            iota_cand_f = constp.tile([128, ncand], F32)
            nc.gpsimd.tensor_copy(iota_cand_f[:], iota_cand_i[:])
            base_tab = constp.tile([128, ncand], F32)   # nloc * (pos >> 3)
            nc.gpsimd.iota(base_tab[:], pattern=[[nloc, ncand // 8], [0, 8]],
                           base=0, channel_multiplier=0,
                           allow_small_or_imprecise_dtypes=True)
            mask_tab = constp.tile([128, ncand], I32)
            nc.gpsimd.memset(mask_tab[:], -(_PACK_MASK + 1))
            iota_m_i = constp.tile([128, nmemb], I32)
            nc.gpsimd.iota(iota_m_i[:], pattern=[[1, nmemb]], base=0,
                           channel_multiplier=0)
            iota_m_f = constp.tile([128, nmemb], F32)
            nc.gpsimd.tensor_copy(iota_m_f[:], iota_m_i[:])

            for qt in range(qtiles):
                r0, r1 = qt * 128, (qt + 1) * 128

                # --- x_norm (exact fp32) ---
                xt = wp.tile([128, D], F32, tag="xt")
                nc.sync.dma_start(out=xt[:], in_=x_d[r0:r1, :])
                xsq = wp.tile([128, D], F32, tag="xsq")
                xn2 = wp.tile([128, 1], F32, tag="xn2")
                nc.scalar.activation(xsq[:], xt[:], ACTF.Square, accum_out=xn2[:])
                xsrt = wp.tile([128, 1], F32, tag="xsrt")
                nc.scalar.activation(xsrt[:], xn2[:], ACTF.Sqrt)
                xinv = wp.tile([128, 1], F32, tag="xinv")
                nc.vector.reciprocal(xinv[:], xsrt[:])
                xn = wp.tile([128, D], F32, tag="xn")
                nc.scalar.activation(xn[:], xt[:], ACTF.Copy, scale=xinv[:])

                # --- candidate tables ---
                vin = wp.tile([128, ncand], F32, tag="vin")
                nc.sync.dma_start(out=vin[:], in_=v_d[r0:r1, :])
                jlow_i = wp.tile([128, ncand], I32, tag="jlowi")
                nc.vector.tensor_scalar(jlow_i[:], vin[:].bitcast(I32),
                                        _PACK_MASK, None, op0=ALU.bitwise_and)
                jlow_f = wp.tile([128, ncand], F32, tag="jlowf")
                nc.vector.tensor_copy(jlow_f[:], jlow_i[:])
                vb = wp.tile([128, ncand], F32, tag="vb")
                nc.vector.tensor_tensor(vb[:].bitcast(I32), vin[:].bitcast(I32),
                                        mask_tab[:], op=ALU.bitwise_and)
                vb2 = wp.tile([128, ncand], F32, tag="vb2")
                nc.vector.tensor_tensor(vb2[:].bitcast(I32), vb[:].bitcast(I32),
                                        iota_cand_i[:], op=ALU.bitwise_or)

                # --- prune to top-`bslots` slots ---
                t12 = wp.tile([128, 16], F32, tag="t12")
                nc.vector.max(out=t12[:, 0:8], in_=vb2[:])
                vrep = wp.tile([128, ncand], F32, tag="vrep")
                nc.vector.match_replace(out=vrep[:], in_to_replace=t12[:, 0:8],
                                        in_values=vb2[:], imm_value=_NEG_BIG)
                nc.vector.max(out=t12[:, 8:16], in_=vrep[:])
                pos_i = wp.tile([128, bslots], I32, tag="posi")
                nc.vector.tensor_scalar(pos_i[:], t12[:, :bslots].bitcast(I32),
                                        ncand - 1, None, op0=ALU.bitwise_and)
                pos_f = wp.tile([128, bslots], F32, tag="posf")
                nc.vector.tensor_copy(pos_f[:], pos_i[:])

                # --- winner slots: j and core-base via one-hot ---
                j_f = wp.tile([128, bslots], F32, tag="jf")
                cb_f = wp.tile([128, bslots], F32, tag="cbf")
                ohtmp = wp.tile([128, ncand], F32, tag="ohtmp")
                ohmul = wp.tile([128, ncand], F32, tag="ohmul")
                for w in range(bslots):
                    nc.vector.tensor_tensor(
                        ohtmp[:], iota_cand_f[:],
                        pos_f[:, w:w + 1].to_broadcast([128, ncand]),
                        op=ALU.is_equal)
                    nc.vector.tensor_tensor(ohmul[:], ohtmp[:], jlow_f[:],
                                            op=ALU.mult)
                    nc.vector.tensor_reduce(j_f[:, w:w + 1], ohmul[:],
                                            axis=AX.X, op=ALU.add)
                    nc.vector.tensor_tensor(ohmul[:], ohtmp[:], base_tab[:],
                                            op=ALU.mult)
                    nc.vector.tensor_reduce(cb_f[:, w:w + 1], ohmul[:],
                                            axis=AX.X, op=ALU.add)

                # --- member rows: base + 2048*(j>>10) + (j&1023) + 1024*m ---
                u_i = wp.tile([128, bslots], I32, tag="ui")
                ji = wp.tile([128, bslots], I32, tag="ji")
                nc.vector.tensor_copy(ji[:], j_f[:])          # f32 -> i32
                nc.vector.tensor_scalar(u_i[:], ji[:], 1023, None,
                                        op0=ALU.bitwise_and)
                u_f = wp.tile([128, bslots], F32, tag="uf")
                nc.vector.tensor_copy(u_f[:], u_i[:])
                bq_f = wp.tile([128, bslots], F32, tag="bqf")   # 2*(j-u) = 2048*b
                nc.vector.tensor_tensor(bq_f[:], j_f[:], u_f[:], op=ALU.subtract)
                rows0 = wp.tile([128, bslots], F32, tag="rows0")
                nc.vector.tensor_scalar(rows0[:], bq_f[:], 2.0, None,
                                        op0=ALU.mult)
                nc.vector.tensor_tensor(rows0[:], rows0[:], u_f[:], op=ALU.add)
                nc.vector.tensor_tensor(rows0[:], rows0[:], cb_f[:], op=ALU.add)

                rows_f = wp.tile([128, nmemb], F32, tag="rowsf")
                for m in range(2):
                    dst = rows_f[:, m * bslots:(m + 1) * bslots]
                    nc.vector.tensor_scalar(dst, rows0[:], float(m * 1024),
                                            None, op0=ALU.add)
                rows_i = wp.tile([128, nmemb], I32, tag="rowsi")
                nc.vector.tensor_copy(rows_i[:], rows_f[:])

                # --- gather member rows + exact rescore ---
                sco = wp.tile([128, nmemb], F32, tag="sco")
                for m in range(2):
                    g = gp.tile([128, bslots, DA], F32, tag="g")
                    for s in range(bslots):
                        nc.gpsimd.indirect_dma_start(
                            out=g[:, s, :], out_offset=None, in_=k_d[:],
                            in_offset=IndirectOffsetOnAxis(
                                ap=rows_i[:, m * bslots + s:m * bslots + s + 1],
                                axis=0))
                    prod = gp.tile([128, bslots, D], F32, tag="prod")
                    xb = xn[:].unsqueeze(1).to_broadcast([128, bslots, D])
                    nc.vector.tensor_tensor(prod[:], g[:, :, :D], xb,
                                            op=ALU.mult)
                    dotm = wp.tile([128, bslots], F32, tag="dotm")
                    nc.vector.tensor_reduce(dotm[:], prod[:], axis=AX.X,
                                            op=ALU.add)
                    nc.vector.tensor_tensor(
                        sco[:, m * bslots:(m + 1) * bslots], dotm[:],
                        g[:, :, D], op=ALU.mult)

                # --- exact top-8 of the members ---
                top8 = wp.tile([128, 8], F32, tag="top8")
                nc.vector.max(out=top8[:], in_=sco[:])
                pos8 = wp.tile([128, 8], U32, tag="pos8")
                nc.vector.max_index(pos8[:], top8[:], sco[:])
                pos8f = wp.tile([128, 8], F32, tag="pos8f")
                nc.vector.tensor_copy(pos8f[:], pos8[:])

                # --- softmax ---
                sh = wp.tile([128, 8], F32, tag="sh")
                nc.vector.tensor_tensor(sh[:], top8[:],
                                        top8[:, 0:1].to_broadcast([128, 8]),
                                        op=ALU.subtract)
                ex = wp.tile([128, 8], F32, tag="ex")
                nc.scalar.activation(ex[:], sh[:], ACTF.Exp)
                es = wp.tile([128, 1], F32, tag="es")
                nc.vector.tensor_reduce(es[:], ex[:], axis=AX.X, op=ALU.add)
                esr = wp.tile([128, 1], F32, tag="esr")
                nc.vector.reciprocal(esr[:], es[:])
                wgt = wp.tile([128, 8], F32, tag="wgt")
                nc.vector.tensor_tensor(wgt[:], ex[:],
                                        esr[:].to_broadcast([128, 8]),
                                        op=ALU.mult)

                # --- winner rows via one-hot over member index ---
                winr = wp.tile([128, 8], F32, tag="winr")
                ohm = wp.tile([128, nmemb], F32, tag="ohm")
                for w in range(8):
                    nc.vector.tensor_tensor(
                        ohm[:], iota_m_f[:],
                        pos8f[:, w:w + 1].to_broadcast([128, nmemb]),
                        op=ALU.is_equal)
                    nc.vector.tensor_tensor(ohm[:], ohm[:], rows_f[:],
                                            op=ALU.mult)
                    nc.vector.tensor_reduce(winr[:, w:w + 1], ohm[:], axis=AX.X,
                                            op=ALU.add)
                winr_i = wp.tile([128, 8], I32, tag="winri")
                nc.vector.tensor_copy(winr_i[:], winr[:])

                # --- gather value rows, weighted sum ---
                vg = gp.tile([128, 8, D], F32, tag="vg")
                for k in range(8):
                    nc.gpsimd.indirect_dma_start(
                        out=vg[:, k, :], out_offset=None, in_=val_d[:],
                        in_offset=IndirectOffsetOnAxis(ap=winr_i[:, k:k + 1],
                                                       axis=0))
                vw = gp.tile([128, 8, D], F32, tag="vw")
                nc.vector.tensor_tensor(
                    vw[:], vg[:],
                    wgt[:].unsqueeze(2).to_broadcast([128, 8, D]), op=ALU.mult)
                ot = wp.tile([128, D], F32, tag="ot")
                nc.vector.tensor_reduce(ot[:], vw[:].rearrange("p k d -> p d k"),
                                        axis=AX.X, op=ALU.add)
                nc.sync.dma_start(out=out_d[r0:r1, :], in_=ot[:])

    nc.compile()
    return nc


# --------------------------------------------------------------------------
# Host orchestration
# --------------------------------------------------------------------------

_CACHE = {}
TRACE = False
last_exec_ns = (None, None)


def _run(nc, in_maps, core_ids):
    if TRACE:
        return run_bass_kernel_spmd(nc, in_maps, core_ids, trace=True)
    return run_bass_kernel_spmd(nc, in_maps, core_ids)


def _get_programs():
    if "A" not in _CACHE:
        _CACHE["A"] = build_dispatch_a()
    if "B" not in _CACHE:
        _CACHE["B"] = build_dispatch_b(B // NCORES)
    return _CACHE["A"], _CACHE["B"]


def kernel(x, keys, values, top_k):
    assert int(top_k) == TOPK
    x = np.ascontiguousarray(np.asarray(x, dtype=np.float32))
    keys = np.asarray(keys, dtype=np.float32)
    values = np.asarray(values, dtype=np.float32)
    assert x.shape == (B, D) and keys.shape == (N, D) and values.shape == (N, D)

    keys_pad = np.zeros((NPAD, D), dtype=np.float32)
    keys_pad[:N] = keys
    values_pad = np.zeros((NPAD, D), dtype=np.float32)
    values_pad[:N] = values

    nc_a, nc_b = _get_programs()
    core_ids = list(range(NCORES))

    # ---- dispatch A ----
    in_maps_a = [
        {"x": x, "keys": np.ascontiguousarray(keys_pad[c * NLOC:(c + 1) * NLOC])}
        for c in range(NCORES)
    ]
    t0 = time.perf_counter()
    res_a = _run(nc_a, in_maps_a, core_ids)
    t1 = time.perf_counter()
    cand = np.concatenate([res_a.results[c]["cand"] for c in range(NCORES)],
                          axis=1)  # [B, 64]
    kinv = np.concatenate([res_a.results[c]["kinv"] for c in range(NCORES)],
                          axis=0)  # [NPAD, 1]
    keys_aug = np.ascontiguousarray(
        np.concatenate([keys_pad, kinv.reshape(NPAD, 1)], axis=1))

    # ---- dispatch B ----
    bs = B // NCORES
    in_maps_b = [
        {
            "vals": np.ascontiguousarray(cand[c * bs:(c + 1) * bs]),
            "x": np.ascontiguousarray(x[c * bs:(c + 1) * bs]),
            "keysaug": keys_aug,
            "values": values_pad,
        }
        for c in range(NCORES)
    ]
    t2 = time.perf_counter()
    res_b = _run(nc_b, in_maps_b, core_ids)
    t3 = time.perf_counter()
    out = np.concatenate([res_b.results[c]["out"] for c in range(NCORES)],
                         axis=0)
    kernel.last_walltimes = (t1 - t0, t3 - t2)
    if TRACE:
        global last_exec_ns
        last_exec_ns = (res_a.exec_time_ns, res_b.exec_time_ns)
    return out.astype(np.float32)



# revision 12
# speedup vs baseline: 1.4072x; 1.4072x over previous
"""Distributed cosine-similarity kNN retrieval (EpisodicSDM) on 8 Trainium2 cores.

Reference computation:
    x_norm = normalize(x); k_norm = normalize(keys)
    scores = x_norm @ k_norm.T               [B, N]
    top_vals, top_idx = top_k(scores, 8)
    out = sum_k softmax(top_vals)_k * values[top_idx_k]

Two SPMD dispatches, no collectives:

Dispatch A (keys sharded along N, all queries everywhere):
  - per 128-key tile: row norms via one fused ACT Square+accumulate, then a
    PE fp32 matmul against diag(1/||k||) = normalized transpose -> kT bf16
  - coarse scores S = x_norm @ k_norm.T in bf16 (fp32 PSUM accumulate),
    26 PSUM n-tiles per 128-query tile
  - fold1 (pairwise max of adjacent PSUM n-tiles): ACT copies the even
    tile to SBUF, DVE tensor_tensor max reads the odd tile from PSUM
    (only one PSUM operand is legal) -> m1 bf16 [128, 6656]
  - fold2 on GPSIMD (adjacent m1 blocks) -> m2 bf16 [128, 3584]
  - ACT widens m2 to fp32, DVE ORs the slot index into the low mantissa
    bits, max8 -> top-8 packed (value, slot) per core, tie-free, no
    max_index pass.
  -> outputs: packed candidates [B, 8] + per-key 1/||k|| [NLOC].

Host glue: concatenate candidates -> [B, 64]; append the device-computed
inverse norms as column 256 of the padded key table (pure data movement).

Dispatch B (queries sharded, 256 per core):
  - re-pack: strip slot bits, embed position 0..63 for tie-free pruning
  - top-12 slots of 64 via max8 + match_replace + max8
  - slot -> 4 member keys: row = core*13312 + 2048*(j>>9) + (j&511) + 512*m
    (leftover fold block: members 2 and 3 are clamped to a pad row)
  - gather member rows [257] (key + inv norm) via indirect DMA
  - exact fp32 rescore s = <x_norm, k> * invnorm_k; top-8 of 48; softmax;
    indirect-gather value rows; weighted sum -> [256, 256] slice.
"""

import os
import sys
import time

_TRN_REPO = "/opt/trn_rl_repo"
if _TRN_REPO not in sys.path:
    sys.path.insert(0, _TRN_REPO)

import numpy as np

import concourse.bass as bass
import concourse.mybir as mybir
import concourse.tile as tile
from concourse import bacc
from concourse.bass import IndirectOffsetOnAxis
from concourse.bass_utils import run_bass_kernel_spmd
from concourse.masks import make_identity

F32 = mybir.dt.float32
BF16 = mybir.dt.bfloat16
I32 = mybir.dt.int32
U32 = mybir.dt.uint32
ALU = mybir.AluOpType
ACTF = mybir.ActivationFunctionType
AX = mybir.AxisListType

# ---- problem constants ----
B = 2048
D = 256
N = 100000
TOPK = 8
NCORES = 8
NLOC = 14336              # 28 * 512 per-core shard; 8*14336 = 114688 >= N
NPAD = NLOC * NCORES
NT = 512
BSLOTS = 12               # slots kept per query after the cross-core merge

_PACK_MASK = 0x1FFF
_NEG_BIG = -3.0e38


def _fold_geom(nloc, nt=NT):
    ntiles = nloc // nt
    assert ntiles % 4 == 0
    ngrp = ntiles // 4           # quad groups: 2048 keys -> 1024 m1 cells
    m1w = ngrp * 2 * nt          # fold output width (nloc / 2)
    return ntiles, ngrp, m1w


# --------------------------------------------------------------------------
# Dispatch A
# --------------------------------------------------------------------------

def build_dispatch_a(bq=B, nloc=NLOC, nt=NT, dbg=False):
    ntiles, ngrp, m1w = _fold_geom(nloc, nt)
    qtiles = bq // 128
    ktiles = nloc // 128
    assert m1w <= _PACK_MASK + 1 and m1w >= 8
    assert ktiles % 4 == 0

    nc = bacc.Bacc("TRN2", target_bir_lowering=False, debug=dbg)
    x_d = nc.dram_tensor("x", [bq, D], F32, kind="ExternalInput").ap()
    k_d = nc.dram_tensor("keys", [nloc, D], F32, kind="ExternalInput").ap()
    out_d = nc.dram_tensor("cand", [bq, 8], F32, kind="ExternalOutput").ap()
    kinv_d = nc.dram_tensor("kinv", [nloc, 1], F32, kind="ExternalOutput").ap()

    with tile.TileContext(nc) as tc:
        with (
            tc.tile_pool(name="const", bufs=1) as constp,
            tc.tile_pool(name="kprep", bufs=4) as kprep,
            tc.tile_pool(name="big", bufs=1) as bigp,
            tc.tile_pool(name="xp", bufs=2) as xp,
            tc.tile_pool(name="sp", bufs=4) as sp,
            tc.tile_pool(name="mp", bufs=1) as mp,
            tc.tile_pool(name="ps", bufs=2, space="PSUM") as psp,
        ):
            identb = constp.tile([128, 128], BF16)
            make_identity(nc, identb[:])
            identf = constp.tile([128, 128], F32)
            make_identity(nc, identf[:])
            eps = constp.tile([128, 1], F32)
            nc.gpsimd.memset(eps[:], 1e-30)
            iota_pack = constp.tile([128, m1w], I32)
            nc.gpsimd.iota(iota_pack[:], pattern=[[1, m1w]], base=0,
                           channel_multiplier=0)
            maskc = constp.tile([128, 1], I32)
            nc.gpsimd.memset(maskc[:], -(_PACK_MASK + 1))

            kT = bigp.tile([128, 2, nloc], BF16)
            xT = bigp.tile([128, 2, bq], BF16)

            # ---- x prep ----
            for qt in range(qtiles):
                xt = xp.tile([128, D], F32, tag="xt")
                nc.sync.dma_start(out=xt[:], in_=x_d[qt * 128:(qt + 1) * 128, :])
                xsq = xp.tile([128, D], F32, tag="xsq")
                xn2 = xp.tile([128, 1], F32, tag="xn2")
                nc.scalar.activation(xsq[:], xt[:], ACTF.Square, accum_out=xn2[:])
                xsrt = xp.tile([128, 1], F32, tag="xsrt")
                nc.scalar.activation(xsrt[:], xn2[:], ACTF.Sqrt)
                xinv = xp.tile([128, 1], F32, tag="xinv")
                nc.vector.reciprocal(xinv[:], xsrt[:])
                xnb = xp.tile([128, D], BF16, tag="xnb")
                nc.scalar.activation(xnb[:], xt[:], ACTF.Copy, scale=xinv[:])
                for c in range(2):
                    pt = psp.tile([128, 2 * nt], F32, tag="pa")
                    nc.tensor.matmul(pt[:, :128], lhsT=xnb[:, c * 128:(c + 1) * 128],
                                     rhs=identb[:], start=True, stop=True)
                    nc.scalar.activation(xT[:, c, qt * 128:(qt + 1) * 128],
                                         pt[:, :128], ACTF.Copy)

            # ---- key prep: groups of 4 tiles -> one PSUM bank per chunk ----
            kinv_all = bigp.tile([128, ktiles], F32)
            for g in range(ktiles // 4):
                ktfs = []
                kn2g = kprep.tile([128, 4], F32, tag="kn2g")
                for i in range(4):
                    tk = g * 4 + i
                    ktf = kprep.tile([128, D], F32, tag=f"ktf{i}")
                    nc.sync.dma_start(out=ktf[:],
                                      in_=k_d[tk * 128:(tk + 1) * 128, :])
                    ksq = kprep.tile([128, D], F32, tag=f"ksq{i}")
                    nc.scalar.activation(ksq[:], ktf[:], ACTF.Square,
                                         accum_out=kn2g[:, i:i + 1])
                    ktfs.append(ktf)
                ksrt = kprep.tile([128, 4], F32, tag="ksrt")
                nc.scalar.activation(ksrt[:], kn2g[:], ACTF.Sqrt, bias=eps[:])
                kinvg = kprep.tile([128, 4], F32, tag="kinvg")
                nc.vector.reciprocal(kinvg[:], ksrt[:])
                nc.vector.tensor_copy(kinv_all[:, g * 4:(g + 1) * 4], kinvg[:])
                diags = []
                for i in range(4):
                    diag = kprep.tile([128, 128], F32, tag=f"diag{i}")
                    nc.gpsimd.tensor_tensor(
                        diag[:], identf[:],
                        kinvg[:, i:i + 1].to_broadcast([128, 128]),
                        op=ALU.mult)
                    diags.append(diag)
                for c in range(2):
                    pt = psp.tile([128, 2 * nt], F32, tag="pa")
                    for i in range(4):
                        nc.tensor.matmul(
                            pt[:, i * 128:(i + 1) * 128],
                            lhsT=ktfs[i][:, c * 128:(c + 1) * 128],
                            rhs=diags[i][:], start=True, stop=True)
                    nc.scalar.activation(kT[:, c, g * nt:(g + 1) * nt],
                                         pt[:, :nt], ACTF.Copy)
            # kinv [128, ktiles] -> DRAM [nloc, 1] (key id = tk*128 + p)
            nc.sync.dma_start(
                out=kinv_d[:].rearrange("(t p) o -> p (t o)", p=128),
                in_=kinv_all[:])

            # ---- main loop: quad groups, single fold to packed fp32 ----
            for qt in range(qtiles):
                m1f = mp.tile([128, m1w], F32, tag="m1f")
                for g in range(ngrp):
                    pa = psp.tile([128, 2 * nt], F32, tag="pa")
                    pb = psp.tile([128, 2 * nt], F32, tag="pb")
                    for idx, pp in enumerate((pa, pa, pb, pb)):
                        nti = 4 * g + idx
                        dst = pp[:, (idx % 2) * nt:(idx % 2 + 1) * nt]
                        for c in range(2):
                            nc.tensor.matmul(
                                dst,
                                lhsT=xT[:, c, qt * 128:(qt + 1) * 128],
                                rhs=kT[:, c, nti * nt:(nti + 1) * nt],
                                start=(c == 0), stop=(c == 1))
                    stmp = sp.tile([128, 2 * nt], F32, tag="stmp")
                    nc.scalar.activation(stmp[:], pa[:], ACTF.Copy)
                    nc.vector.tensor_tensor(
                        m1f[:, g * 2 * nt:(g + 1) * 2 * nt],
                        pb[:], stmp[:], op=ALU.max)
                nc.vector.scalar_tensor_tensor(
                    m1f[:].bitcast(I32), m1f[:].bitcast(I32),
                    maskc[:], iota_pack[:],
                    op0=ALU.bitwise_and, op1=ALU.bitwise_or)
                top = mp.tile([128, 8], F32, tag="top")
                nc.vector.max(out=top[:], in_=m1f[:])
                nc.sync.dma_start(out=out_d[qt * 128:(qt + 1) * 128, :],
                                  in_=top[:])

    nc.compile()
    return nc


# --------------------------------------------------------------------------
# Dispatch B
# --------------------------------------------------------------------------

def build_dispatch_b(bq_slice, nloc=NLOC, npad=NPAD, ncand=NCORES * 8,
                     bslots=BSLOTS, dbg=False):
    qtiles = bq_slice // 128
    ntiles, ngrp, m1w = _fold_geom(nloc)
    nmemb = bslots * 2
    DA = D + 1   # augmented row: 256 dims + inv-norm

    nc = bacc.Bacc("TRN2", target_bir_lowering=False, debug=dbg)
    v_d = nc.dram_tensor("vals", [bq_slice, ncand], F32, kind="ExternalInput").ap()
    x_d = nc.dram_tensor("x", [bq_slice, D], F32, kind="ExternalInput").ap()
    k_d = nc.dram_tensor("keysaug", [npad, DA], F32, kind="ExternalInput").ap()
    val_d = nc.dram_tensor("values", [npad, D], F32, kind="ExternalInput").ap()
    out_d = nc.dram_tensor("out", [bq_slice, D], F32, kind="ExternalOutput").ap()

    with tile.TileContext(nc) as tc:
        with (
            tc.tile_pool(name="const", bufs=1) as constp,
            tc.tile_pool(name="wp", bufs=2) as wp,
            tc.tile_pool(name="gp", bufs=2) as gp,
        ):
            iota_cand_i = constp.tile([128, ncand], I32)
            nc.gpsimd.iota(iota_cand_i[:], pattern=[[1, ncand]], base=0,
                           channel_multiplier=0)
            iota_cand_f = constp.tile([128, ncand], F32)
            nc.gpsimd.tensor_copy(iota_cand_f[:], iota_cand_i[:])
            base_tab = constp.tile([128, ncand], F32)   # nloc * (pos >> 3)
            nc.gpsimd.iota(base_tab[:], pattern=[[nloc, ncand // 8], [0, 8]],
                           base=0, channel_multiplier=0,
                           allow_small_or_imprecise_dtypes=True)
            mask_tab = constp.tile([128, ncand], I32)
            nc.gpsimd.memset(mask_tab[:], -(_PACK_MASK + 1))
            iota_m_i = constp.tile([128, nmemb], I32)
            nc.gpsimd.iota(iota_m_i[:], pattern=[[1, nmemb]], base=0,
                           channel_multiplier=0)
            iota_m_f = constp.tile([128, nmemb], F32)
            nc.gpsimd.tensor_copy(iota_m_f[:], iota_m_i[:])

            for qt in range(qtiles):
                r0, r1 = qt * 128, (qt + 1) * 128

                # --- x_norm (exact fp32) ---
                xt = wp.tile([128, D], F32, tag="xt")
                nc.sync.dma_start(out=xt[:], in_=x_d[r0:r1, :])
                xsq = wp.tile([128, D], F32, tag="xsq")
                xn2 = wp.tile([128, 1], F32, tag="xn2")
                nc.scalar.activation(xsq[:], xt[:], ACTF.Square, accum_out=xn2[:])
                xsrt = wp.tile([128, 1], F32, tag="xsrt")
                nc.scalar.activation(xsrt[:], xn2[:], ACTF.Sqrt)
                xinv = wp.tile([128, 1], F32, tag="xinv")
                nc.vector.reciprocal(xinv[:], xsrt[:])
                xn = wp.tile([128, D], F32, tag="xn")
                nc.scalar.activation(xn[:], xt[:], ACTF.Copy, scale=xinv[:])

                # --- candidate tables ---
                vin = wp.tile([128, ncand], F32, tag="vin")
                nc.sync.dma_start(out=vin[:], in_=v_d[r0:r1, :])
                jlow_i = wp.tile([128, ncand], I32, tag="jlowi")
                nc.vector.tensor_scalar(jlow_i[:], vin[:].bitcast(I32),
                                        _PACK_MASK, None, op0=ALU.bitwise_and)
                jlow_f = wp.tile([128, ncand], F32, tag="jlowf")
                nc.vector.tensor_copy(jlow_f[:], jlow_i[:])
                vb = wp.tile([128, ncand], F32, tag="vb")
                nc.vector.tensor_tensor(vb[:].bitcast(I32), vin[:].bitcast(I32),
                                        mask_tab[:], op=ALU.bitwise_and)
                vb2 = wp.tile([128, ncand], F32, tag="vb2")
                nc.vector.tensor_tensor(vb2[:].bitcast(I32), vb[:].bitcast(I32),
                                        iota_cand_i[:], op=ALU.bitwise_or)

                # --- prune to top-`bslots` slots ---
                t12 = wp.tile([128, 16], F32, tag="t12")
                nc.vector.max(out=t12[:, 0:8], in_=vb2[:])
                vrep = wp.tile([128, ncand], F32, tag="vrep")
                nc.vector.match_replace(out=vrep[:], in_to_replace=t12[:, 0:8],
                                        in_values=vb2[:], imm_value=_NEG_BIG)
                nc.vector.max(out=t12[:, 8:16], in_=vrep[:])
                pos_i = wp.tile([128, bslots], I32, tag="posi")
                nc.vector.tensor_scalar(pos_i[:], t12[:, :bslots].bitcast(I32),
                                        ncand - 1, None, op0=ALU.bitwise_and)
                pos_f = wp.tile([128, bslots], F32, tag="posf")
                nc.vector.tensor_copy(pos_f[:], pos_i[:])

                # --- winner slots: j and core-base via one-hot ---
                j_f = wp.tile([128, bslots], F32, tag="jf")
                cb_f = wp.tile([128, bslots], F32, tag="cbf")
                ohtmp = wp.tile([128, ncand], F32, tag="ohtmp")
                ohmul = wp.tile([128, ncand], F32, tag="ohmul")
                for w in range(bslots):
                    nc.vector.tensor_tensor(
                        ohtmp[:], iota_cand_f[:],
                        pos_f[:, w:w + 1].to_broadcast([128, ncand]),
                        op=ALU.is_equal)
                    nc.vector.tensor_tensor(ohmul[:], ohtmp[:], jlow_f[:],
                                            op=ALU.mult)
                    nc.vector.tensor_reduce(j_f[:, w:w + 1], ohmul[:],
                                            axis=AX.X, op=ALU.add)
                    nc.vector.tensor_tensor(ohmul[:], ohtmp[:], base_tab[:],
                                            op=ALU.mult)
                    nc.vector.tensor_reduce(cb_f[:, w:w + 1], ohmul[:],
                                            axis=AX.X, op=ALU.add)

                # --- member rows: base + 2048*(j>>10) + (j&1023) + 1024*m ---
                u_i = wp.tile([128, bslots], I32, tag="ui")
                ji = wp.tile([128, bslots], I32, tag="ji")
                nc.vector.tensor_copy(ji[:], j_f[:])          # f32 -> i32
                nc.vector.tensor_scalar(u_i[:], ji[:], 1023, None,
                                        op0=ALU.bitwise_and)
                u_f = wp.tile([128, bslots], F32, tag="uf")
                nc.vector.tensor_copy(u_f[:], u_i[:])
                bq_f = wp.tile([128, bslots], F32, tag="bqf")   # 2*(j-u) = 2048*b
                nc.vector.tensor_tensor(bq_f[:], j_f[:], u_f[:], op=ALU.subtract)
                rows0 = wp.tile([128, bslots], F32, tag="rows0")
                nc.vector.tensor_scalar(rows0[:], bq_f[:], 2.0, None,
                                        op0=ALU.mult)
                nc.vector.tensor_tensor(rows0[:], rows0[:], u_f[:], op=ALU.add)
                nc.vector.tensor_tensor(rows0[:], rows0[:], cb_f[:], op=ALU.add)

                rows_f = wp.tile([128, nmemb], F32, tag="rowsf")
                for m in range(2):
                    dst = rows_f[:, m * bslots:(m + 1) * bslots]
                    nc.vector.tensor_scalar(dst, rows0[:], float(m * 1024),
                                            None, op0=ALU.add)
                rows_i = wp.tile([128, nmemb], I32, tag="rowsi")
                nc.vector.tensor_copy(rows_i[:], rows_f[:])

                # --- gather member rows + exact rescore ---
                sco = wp.tile([128, nmemb], F32, tag="sco")
                for m in range(2):
                    g = gp.tile([128, bslots, DA], F32, tag="g")
                    for s in range(bslots):
                        nc.gpsimd.indirect_dma_start(
                            out=g[:, s, :], out_offset=None, in_=k_d[:],
                            in_offset=IndirectOffsetOnAxis(
                                ap=rows_i[:, m * bslots + s:m * bslots + s + 1],
                                axis=0))
                    prod = gp.tile([128, bslots, D], F32, tag="prod")
                    xb = xn[:].unsqueeze(1).to_broadcast([128, bslots, D])
                    nc.vector.tensor_tensor(prod[:], g[:, :, :D], xb,
                                            op=ALU.mult)
                    dotm = wp.tile([128, bslots], F32, tag="dotm")
                    nc.vector.tensor_reduce(dotm[:], prod[:], axis=AX.X,
                                            op=ALU.add)
                    nc.vector.tensor_tensor(
                        sco[:, m * bslots:(m + 1) * bslots], dotm[:],
                        g[:, :, D], op=ALU.mult)

                # --- exact top-8 of the members ---
                top8 = wp.tile([128, 8], F32, tag="top8")
                nc.vector.max(out=top8[:], in_=sco[:])
                pos8 = wp.tile([128, 8], U32, tag="pos8")
                nc.vector.max_index(pos8[:], top8[:], sco[:])
                pos8f = wp.tile([128, 8], F32, tag="pos8f")
                nc.vector.tensor_copy(pos8f[:], pos8[:])

                # --- softmax ---
                sh = wp.tile([128, 8], F32, tag="sh")
                nc.vector.tensor_tensor(sh[:], top8[:],
                                        top8[:, 0:1].to_broadcast([128, 8]),
                                        op=ALU.subtract)
                ex = wp.tile([128, 8], F32, tag="ex")
                nc.scalar.activation(ex[:], sh[:], ACTF.Exp)
                es = wp.tile([128, 1], F32, tag="es")
                nc.vector.tensor_reduce(es[:], ex[:], axis=AX.X, op=ALU.add)
                esr = wp.tile([128, 1], F32, tag="esr")
                nc.vector.reciprocal(esr[:], es[:])
                wgt = wp.tile([128, 8], F32, tag="wgt")
                nc.vector.tensor_tensor(wgt[:], ex[:],
                                        esr[:].to_broadcast([128, 8]),
                                        op=ALU.mult)

                # --- winner rows via one-hot over member index ---
                winr = wp.tile([128, 8], F32, tag="winr")
                ohm = wp.tile([128, nmemb], F32, tag="ohm")
                for w in range(8):
                    nc.vector.tensor_tensor(
                        ohm[:], iota_m_f[:],
                        pos8f[:, w:w + 1].to_broadcast([128, nmemb]),
                        op=ALU.is_equal)
                    nc.vector.tensor_tensor(ohm[:], ohm[:], rows_f[:],
                                            op=ALU.mult)
                    nc.vector.tensor_reduce(winr[:, w:w + 1], ohm[:], axis=AX.X,
                                            op=ALU.add)
                winr_i = wp.tile([128, 8], I32, tag="winri")
                nc.vector.tensor_copy(winr_i[:], winr[:])

                # --- gather value rows, weighted sum ---
                vg = gp.tile([128, 8, D], F32, tag="vg")
                for k in range(8):
                    nc.gpsimd.indirect_dma_start(
                        out=vg[:, k, :], out_offset=None, in_=val_d[:],
                        in_offset=IndirectOffsetOnAxis(ap=winr_i[:, k:k + 1],
                                                       axis=0))
                vw = gp.tile([128, 8, D], F32, tag="vw")
                nc.vector.tensor_tensor(
                    vw[:], vg[:],
                    wgt[:].unsqueeze(2).to_broadcast([128, 8, D]), op=ALU.mult)
                ot = wp.tile([128, D], F32, tag="ot")
                nc.vector.tensor_reduce(ot[:], vw[:].rearrange("p k d -> p d k"),
                                        axis=AX.X, op=ALU.add)
                nc.sync.dma_start(out=out_d[r0:r1, :], in_=ot[:])

    nc.compile()
    return nc


# --------------------------------------------------------------------------
# Host orchestration
# --------------------------------------------------------------------------

_CACHE = {}
TRACE = False
last_exec_ns = (None, None)


def _run(nc, in_maps, core_ids):
    if TRACE:
        return run_bass_kernel_spmd(nc, in_maps, core_ids, trace=True)
    return run_bass_kernel_spmd(nc, in_maps, core_ids)


def _get_programs():
    if "A" not in _CACHE:
        _CACHE["A"] = build_dispatch_a()
    if "B" not in _CACHE:
        _CACHE["B"] = build_dispatch_b(B // NCORES)
    return _CACHE["A"], _CACHE["B"]


def kernel(x, keys, values, top_k):
    assert int(top_k) == TOPK
    x = np.ascontiguousarray(np.asarray(x, dtype=np.float32))
    keys = np.asarray(keys, dtype=np.float32)
    values = np.asarray(values, dtype=np.float32)
    assert x.shape == (B, D) and keys.shape == (N, D) and values.shape == (N, D)

    keys_pad = np.zeros((NPAD, D), dtype=np.float32)
    keys_pad[:N] = keys
    values_pad = np.zeros((NPAD, D), dtype=np.float32)
    values_pad[:N] = values

    nc_a, nc_b = _get_programs()
    core_ids = list(range(NCORES))

    # ---- dispatch A ----
    in_maps_a = [
        {"x": x, "keys": np.ascontiguousarray(keys_pad[c * NLOC:(c + 1) * NLOC])}
        for c in range(NCORES)
    ]
    t0 = time.perf_counter()
    res_a = _run(nc_a, in_maps_a, core_ids)
    t1 = time.perf_counter()
    cand = np.concatenate([res_a.results[c]["cand"] for c in range(NCORES)],
                          axis=1)  # [B, 64]
    kinv = np.concatenate([res_a.results[c]["kinv"] for c in range(NCORES)],
                          axis=0)  # [NPAD, 1]
    keys_aug = np.ascontiguousarray(
        np.concatenate([keys_pad, kinv.reshape(NPAD, 1)], axis=1))

    # ---- dispatch B ----
    bs = B // NCORES
    in_maps_b = [
        {
            "vals": np.ascontiguousarray(cand[c * bs:(c + 1) * bs]),
            "x": np.ascontiguousarray(x[c * bs:(c + 1) * bs]),
            "keysaug": keys_aug,
            "values": values_pad,
        }
        for c in range(NCORES)
    ]
    t2 = time.perf_counter()
    res_b = _run(nc_b, in_maps_b, core_ids)
    t3 = time.perf_counter()
    out = np.concatenate([res_b.results[c]["out"] for c in range(NCORES)],
                         axis=0)
    kernel.last_walltimes = (t1 - t0, t3 - t2)
    if TRACE:
        global last_exec_ns
        last_exec_ns = (res_a.exec_time_ns, res_b.exec_time_ns)
    return out.astype(np.float32)

